# revision 11
# baseline (speedup 1.0000x reference)
"""Trainium2 Bass kernel for nn_DiscoBoxv2Head (matrix-NMS + mean-field CRF).

Self-contained: hardcodes shapes/sharding for the fixed problem size
  seg_masks (256,128,128) f32, cate_scores (256,) f32,
  feature_map (64,3,128,128) f32, x/targets (64,1,128,128) f32,
  cate_labels (256,) int64.

Sharding over 8 cores:
  - CRF: data-parallel over the 64 instances (8 per core).
  - NMS: contraction (K)-sharded m@m.T -> per-core partial (256,257)
    [inter | row-sums] -> AllReduce(add) -> epilogue redundantly per core.

CRF math (exact reformulation of the reference):
  ret only takes values {0.45, 0.55}, so each mean-field iteration is the
  binary update  m <- targets * (sum_o k_o * shift_o(m) > Ksum/2)  with
  k_o = exp(-2*sum_c(dg_c^2) - s_o/1800) (ALPHA0 drops out), k_center = 1,
  and k_{-o}(p) = k_o(p-delta_o).  Zero padding of g = fm+10 makes k
  underflow to exactly 0 at image borders, so shifts can read garbage-free
  zero pads.  Verified bit-identical masks vs the jax reference.
"""
from contextlib import ExitStack

import numpy as np
import ml_dtypes

import concourse.bass as bass
import concourse.bacc as bacc
import concourse.tile as tile
from concourse import mybir
from concourse.bass_utils import run_bass_kernel_spmd

F32 = mybir.dt.float32
BF16 = mybir.dt.bfloat16
AX = mybir.AxisListType.X
OP = mybir.AluOpType
AF = mybir.ActivationFunctionType

NCORES = 8
NI = 8              # instances per core
H = W = 128
STR = 130           # strip stride (1 pad + 128 + 1 pad)
WT = NI * STR + 2   # 1042 tile width
LO, HI = 1, WT - 1  # elementwise compute window [1, 1041)
KC = 2048           # NMS contraction chunk per core
N = 256             # NMS candidates
BIG = 1.0e30

# strip i occupies cols [STR*i+2, STR*i+130)
def _strip(i):
    return slice(STR * i + 2, STR * i + 2 + W)


def _win(t, fo=0):
    """AP over the compute window with free offset fo."""
    return t[:, LO + fo:HI + fo]


def _build_nc():
    nc = bacc.Bacc("TRN2", target_bir_lowering=False, debug=False, num_devices=8)
    # ---- DRAM I/O ----
    g_in = nc.declare_dram_parameter("g", [3, H, WT], F32, isOutput=False)
    xp_in = nc.declare_dram_parameter("xp", [H, WT], F32, isOutput=False)
    tp_in = nc.declare_dram_parameter("tp", [H, WT], F32, isOutput=False)
    segT_in = nc.declare_dram_parameter("segT", [KC, N + 1], BF16, isOutput=False)
    maskT_in = nc.declare_dram_parameter("maskT", [N, N], F32, isOutput=False)
    scor_in = nc.declare_dram_parameter("scores_col", [N, 1], F32, isOutput=False)
    shp1_in = nc.declare_dram_parameter("sh_p1", [H, H], F32, isOutput=False)
    shm1_in = nc.declare_dram_parameter("sh_m1", [H, H], F32, isOutput=False)
    iden_in = nc.declare_dram_parameter("ident", [H, H], F32, isOutput=False)
    ones_in = nc.declare_dram_parameter("ones_row", [1, H], F32, isOutput=False)

    masks_out = nc.declare_dram_parameter("masks", [H, NI * W], F32, isOutput=True)
    valid_out = nc.declare_dram_parameter("valid", [1, NI], F32, isOutput=True)
    scores_out = nc.declare_dram_parameter("scores", [1, N], F32, isOutput=True)

    cc_in = nc.dram_tensor("cc_in", [2 * H, N + 1], F32)
    cc_out = nc.dram_tensor("cc_out", [2 * H, N + 1], F32)

    # PE chunk windows (with 2-col overlap) covering [0, WT)
    CH = [(0, 262), (260, 522), (520, 782), (780, 1042)]

    with tile.TileContext(nc, num_cores=NCORES) as tc:
        with (
            tc.tile_pool(name="const", bufs=1) as constp,
            tc.tile_pool(name="kf", bufs=1) as kfp,
            tc.tile_pool(name="mst", bufs=1) as mstp,
            tc.tile_pool(name="psum", bufs=1, space="PSUM") as psp,
            tc.tile_pool(name="small", bufs=1) as smallp,
        ):
            # ---------- constants ----------
            ident = constp.tile([H, H], F32, tag="ident")
            nc.sync.dma_start(out=ident, in_=iden_in[:, :])
            ones_row = constp.tile([1, H], F32, tag="ones_row")
            nc.sync.dma_start(out=ones_row, in_=ones_in[:, :])
            sh_p1 = constp.tile([H, H], F32, tag="sh_p1")
            nc.sync.dma_start(out=sh_p1, in_=shp1_in[:, :])
            sh_m1 = constp.tile([H, H], F32, tag="sh_m1")
            nc.sync.dma_start(out=sh_m1, in_=shm1_in[:, :])
            maskT = [constp.tile([H, N], F32, tag=f"maskT{b}", name=f"maskT{b}") for b in range(2)]
            for b in range(2):
                nc.sync.dma_start(out=maskT[b], in_=maskT_in[b * H:(b + 1) * H, :])
            scol = [constp.tile([H, 1], F32, tag=f"scol{b}", name=f"scol{b}") for b in range(2)]
            for b in range(2):
                nc.sync.dma_start(out=scol[b], in_=scor_in[b * H:(b + 1) * H, :])
            tp = constp.tile([H, WT], F32, tag="tp")
            nc.sync.dma_start(out=tp, in_=tp_in[:, :])
            bias_z = constp.tile([H, 1], F32, tag="bias_z")
            nc.vector.memset(bias_z, 0.0)
            bias_s = [constp.tile([H, 1], F32, tag=f"bias_s{j}", name=f"bias_s{j}") for j in (1, 2)]
            nc.vector.memset(bias_s[0], -1.0 / 1800.0)
            nc.vector.memset(bias_s[1], -2.0 / 1800.0)

            # ---------- NMS local GEMM ----------
            with (tc.tile_pool(name="seg", bufs=16) as segp,
                  tc.tile_pool(name="psnms", bufs=1, space="PSUM") as psnms):
                st = []
                for kt in range(16):
                    s = segp.tile([H, N + 1], BF16, tag="st", name=f"st{kt}")
                    nc.sync.dma_start(out=s, in_=segT_in[kt * H:(kt + 1) * H, :])
                    st.append(s)
                pn = [psnms.tile([H, N + 1], F32, tag=f"pn{b}", name=f"pn{b}") for b in range(2)]
                for ib in range(2):
                    for kt in range(16):
                        nc.tensor.matmul(
                            pn[ib], lhsT=st[kt][:, ib * H:(ib + 1) * H],
                            rhs=st[kt][:, :], start=(kt == 0), stop=(kt == 15))
                inter_l = [smallp.tile([H, N + 1], F32, tag=f"interl{b}", name=f"interl{b}") for b in range(2)]
                for b in range(2):
                    nc.scalar.copy(inter_l[b], pn[b])
                    nc.sync.dma_start(out=cc_in[b * H:(b + 1) * H, :], in_=inter_l[b])
            nc.gpsimd.collective_compute(
                "AllReduce", OP.add,
                replica_groups=[list(range(NCORES))],
                ins=[cc_in[:, :].opt()], outs=[cc_out[:, :].opt()])
            inter = [constp.tile([H, N + 1], F32, tag=f"inter{b}", name=f"inter{b}") for b in range(2)]
            for b in range(2):
                nc.sync.dma_start(out=inter[b], in_=cc_out[b * H:(b + 1) * H, :])

            # ---------- CRF precompute: kernel fields ----------
            kE = kfp.tile([H, WT], F32, tag="kE")
            kSE = kfp.tile([H, WT], F32, tag="kSE")
            kS = kfp.tile([H, WT], F32, tag="kS")
            kSW = kfp.tile([H, WT], F32, tag="kSW")
            kSEu = kfp.tile([H, WT], F32, tag="kSEu")
            kSu = kfp.tile([H, WT], F32, tag="kSu")
            kSWu = kfp.tile([H, WT], F32, tag="kSWu")
            for t in (kE, kSE, kS, kSW):
                nc.vector.memset(t[:, 0:1], 0.0)
                nc.vector.memset(t[:, WT - 1:WT], 0.0)

            with tc.tile_pool(name="pre", bufs=1) as prep:
                g = []
                for c in range(3):
                    gt = prep.tile([H, WT], F32, tag=f"g{c}", name=f"g{c}")
                    nc.sync.dma_start(out=gt, in_=g_in[c, :, :])
                    g.append(gt)
                # gds[c] = g_c shifted: gds(i,j) = g_c(i+1, j)
                gds = []
                for c in range(3):
                    gd = prep.tile([H, WT], F32, tag=f"gd{c}", name=f"gd{c}")
                    for (a, bnd) in CH:
                        pch = psp.tile([H, 262], F32, tag="chunk", name="pch", bufs=4)
                        nc.tensor.matmul(pch[:, 0:bnd - a], lhsT=sh_p1,
                                         rhs=g[c][:, a:bnd], start=True, stop=True)
                        nc.scalar.copy(gd[:, a:bnd], pch[:, 0:bnd - a])
                    gds.append(gd)

                # field defs: (name, ktile, src fn, free offset, spatial s)
                fields = [
                    ("E", kE, g, 1, 1.0),
                    ("SE", kSE, gds, 1, 2.0),
                    ("S", kS, gds, 0, 1.0),
                    ("SW", kSW, gds, -1, 2.0),
                ]
                eng_d = [nc.vector, nc.gpsimd]
                di = 0
                for fi, (nm, ktile, src, fo, s) in enumerate(fields):
                    sqs = []
                    for c in range(3):
                        d = prep.tile([H, WT], F32, tag="d", name="d", bufs=3)
                        e = eng_d[di % 2]; di += 1
                        e.tensor_tensor(_win(d), _win(src[c], fo), _win(g[c]),
                                        OP.subtract)
                        sq = prep.tile([H, WT], F32, tag="sq", name="sq", bufs=3)
                        nc.scalar.activation(_win(sq), _win(d), AF.Square, bias=bias_z)
                        sqs.append(sq)
                    ss = prep.tile([H, WT], F32, tag="ss", bufs=2)
                    e = eng_d[di % 2]; di += 1
                    e.tensor_tensor(_win(ss), _win(sqs[0]), _win(sqs[1]), OP.add)
                    e = eng_d[di % 2]; di += 1
                    e.tensor_tensor(_win(ss), _win(ss), _win(sqs[2]), OP.add)
                    nc.scalar.activation(_win(ktile), _win(ss), AF.Exp,
                                         bias=bias_s[int(s) - 1], scale=-2.0)

                # opposite-direction fields via PE row-shift up
                for ksrc, kdst in ((kSE, kSEu), (kS, kSu), (kSW, kSWu)):
                    for (a, bnd) in CH:
                        pch = psp.tile([H, 262], F32, tag="chunk", name="pch", bufs=4)
                        nc.tensor.matmul(pch[:, 0:bnd - a], lhsT=sh_m1,
                                         rhs=ksrc[:, a:bnd], start=True, stop=True)
                        nc.scalar.copy(kdst[:, a:bnd], pch[:, 0:bnd - a])

                # Ksum -> threshold tile (with target gate folded in)
                ha = prep.tile([H, WT], F32, tag="ha")
                hb = prep.tile([H, WT], F32, tag="hb")
                hc = prep.tile([H, WT], F32, tag="hc")
                hd = prep.tile([H, WT], F32, tag="hd")
                nc.vector.tensor_tensor(_win(ha), _win(kE), _win(kE, -1), OP.add)
                nc.gpsimd.tensor_tensor(_win(hb), _win(kSE), _win(kSEu, -1), OP.add)
                nc.vector.tensor_tensor(_win(hc), _win(kS), _win(kSu), OP.add)
                nc.gpsimd.tensor_tensor(_win(hd), _win(kSW), _win(kSWu, 1), OP.add)
                nc.vector.tensor_tensor(_win(ha), _win(ha), _win(hb), OP.add)
                nc.gpsimd.tensor_tensor(_win(hc), _win(hc), _win(hd), OP.add)
                nc.vector.tensor_tensor(_win(ha), _win(ha), _win(hc), OP.add)
                kthr = mstp.tile([H, WT], F32, tag="kthr")
                nc.gpsimd.memset(kthr, BIG)
                kpre = prep.tile([H, WT], F32, tag="kpre")
                nc.vector.tensor_scalar(_win(kpre), _win(ha), 1.0, 0.5,
                                        OP.add, OP.mult)
                # gate = (1-tp)*BIG in {0, BIG}; kthr = kpre + gate is exact
                # where tp==1 and a huge threshold (mask stays 0) where tp==0
                gate = prep.tile([H, WT], F32, tag="gate")
                nc.vector.tensor_scalar(_win(gate), _win(tp), -BIG, BIG,
                                        OP.mult, OP.add)
                nc.vector.tensor_tensor(_win(kthr), _win(kpre), _win(gate), OP.add)

                # m0 = (x*t > 0.5)
                mA = mstp.tile([H, WT], F32, tag="mA")
                mB = mstp.tile([H, WT], F32, tag="mB")
                nc.gpsimd.memset(mA, 0.0)
                nc.gpsimd.memset(mB, 0.0)
                xt = prep.tile([H, WT], F32, tag="xt")
                xp = prep.tile([H, WT], F32, tag="xp")
                nc.sync.dma_start(out=xp, in_=xp_in[:, :])
                nc.vector.tensor_tensor(_win(xt), _win(xp), _win(tp), OP.mult)
                nc.vector.tensor_scalar(_win(mA), _win(xt), 0.5, None, OP.is_gt)

            # ---------- CRF iterations ----------
            iterstack = ExitStack()
            prodp = iterstack.enter_context(tc.tile_pool(name="prod", bufs=8))
            accp = iterstack.enter_context(tc.tile_pool(name="accs", bufs=2))
            mshp = iterstack.enter_context(tc.tile_pool(name="mshift", bufs=2))
            mtiles = [mA, mB]
            for it in range(10):
                mc = mtiles[it % 2]
                mn = mtiles[(it + 1) % 2]
                mdn = mshp.tile([H, WT], F32, tag="mdn")
                mup = mshp.tile([H, WT], F32, tag="mup")
                for sh, dst in ((sh_p1, mdn), (sh_m1, mup)):
                    for (a, bnd) in CH:
                        pch = psp.tile([H, 262], F32, tag="chunk", name="pch", bufs=4)
                        nc.tensor.matmul(pch[:, 0:bnd - a], lhsT=sh,
                                         rhs=mc[:, a:bnd], start=True, stop=True)
                        nc.scalar.copy(dst[:, a:bnd], pch[:, 0:bnd - a])
                P = [prodp.tile([H, WT], F32, tag="P", name=f"P{j}") for j in range(8)]
                nc.vector.tensor_tensor(_win(P[0]), _win(kE), _win(mc, 1), OP.mult)
                nc.vector.tensor_tensor(_win(P[1]), _win(kE, -1), _win(mc, -1), OP.mult)
                nc.vector.tensor_tensor(_win(P[2]), _win(kSE), _win(mdn, 1), OP.mult)
                nc.vector.tensor_tensor(_win(P[3]), _win(kS), _win(mdn), OP.mult)
                nc.gpsimd.tensor_tensor(_win(P[4]), _win(kSW), _win(mdn, -1), OP.mult)
                nc.gpsimd.tensor_tensor(_win(P[5]), _win(kSEu, -1), _win(mup, -1), OP.mult)
                nc.gpsimd.tensor_tensor(_win(P[6]), _win(kSu), _win(mup), OP.mult)
                nc.gpsimd.tensor_tensor(_win(P[7]), _win(kSWu, 1), _win(mup, 1), OP.mult)
                A = accp.tile([H, WT], F32, tag="A")
                B = accp.tile([H, WT], F32, tag="B")
                C = accp.tile([H, WT], F32, tag="C")
                D = accp.tile([H, WT], F32, tag="D")
                nc.vector.tensor_tensor(_win(A), _win(P[0]), _win(P[1]), OP.add)
                nc.vector.tensor_tensor(_win(B), _win(P[2]), _win(P[3]), OP.add)
                nc.gpsimd.tensor_tensor(_win(C), _win(P[4]), _win(P[5]), OP.add)
                nc.gpsimd.tensor_tensor(_win(D), _win(P[6]), _win(P[7]), OP.add)
                nc.vector.tensor_tensor(_win(A), _win(A), _win(B), OP.add)
                nc.vector.tensor_tensor(_win(C), _win(C), _win(D), OP.add)
                nc.vector.tensor_tensor(_win(A), _win(A), _win(mc), OP.add)
                nc.vector.tensor_tensor(_win(A), _win(A), _win(C), OP.add)
                nc.vector.tensor_tensor(_win(mn), _win(A), _win(kthr), OP.is_gt)
            psep = iterstack.enter_context(
                tc.tile_pool(name="psep", bufs=1, space="PSUM"))
            mfin = mtiles[0]

            # ---------- outputs: masks + valid ----------
            for i in range(NI):
                nc.sync.dma_start(out=masks_out[:, i * W:(i + 1) * W],
                                  in_=mfin[:, _strip(i)])
            rs = smallp.tile([H, NI], F32, tag="rs")
            mv = bass.AP(tensor=mfin.tensor, offset=mfin.offset + 2,
                         ap=[mfin.ap[0], [STR, NI], [1, W]])
            nc.vector.tensor_reduce(rs, mv, AX, OP.add)
            ones_col = smallp.tile([H, 1], F32, tag="ones_col")
            nc.vector.memset(ones_col, 1.0)
            pcnt = psep.tile([1, NI], F32, tag="pcnt")
            nc.tensor.matmul(pcnt, lhsT=ones_col, rhs=rs, start=True, stop=True)
            va = smallp.tile([1, NI], F32, tag="va")
            vb = smallp.tile([1, NI], F32, tag="vb")
            nc.vector.tensor_scalar(va, pcnt, float(H * W * 0.05), None, OP.is_ge)
            nc.vector.tensor_scalar(vb, pcnt, float(H * W * 0.95), None, OP.is_le)
            nc.vector.tensor_tensor(va, va, vb, OP.mult)
            nc.sync.dma_start(out=valid_out[:, :], in_=va)

            # ---------- NMS epilogue ----------
            # layout [a(part)=j, b(free)=i]; cell(a,b) = -decay_iou[b,a]
            srow = smallp.tile([1, N], F32, tag="srow")
            for b in range(2):
                pr = psep.tile([1, H], F32, tag="prow", name="prow")
                nc.tensor.matmul(pr, lhsT=inter[b][:, N:N + 1], rhs=ident,
                                 start=True, stop=True)
                nc.scalar.copy(srow[:, b * H:(b + 1) * H], pr)
            psb = psep.tile([H, N], F32, tag="psb")
            nc.tensor.matmul(psb, lhsT=ones_row, rhs=srow, start=True, stop=True)
            cell = [smallp.tile([H, N], F32, tag=f"cell{b}", name=f"cell{b}") for b in range(2)]
            rmin = [smallp.tile([H, 1], F32, tag=f"rmin{b}", name=f"rmin{b}") for b in range(2)]
            decay = [smallp.tile([H, N], F32, tag=f"decay{b}", name=f"decay{b}") for b in range(2)]
            for b in range(2):
                nu = smallp.tile([H, N], F32, tag=f"nu{b}", name=f"nu{b}")
                # nu = (inter - s_a) - s_b  == -union
                nc.vector.scalar_tensor_tensor(nu, inter[b][:, 0:N],
                                               inter[b][:, N:N + 1], psb,
                                               OP.subtract, OP.subtract)
                q = smallp.tile([H, N], F32, tag=f"q{b}", name=f"q{b}")
                nc.vector.tensor_tensor(q, inter[b][:, 0:N], maskT[b], OP.mult)
                rnu = smallp.tile([H, N], F32, tag=f"rnu{b}", name=f"rnu{b}")
                nc.vector.reciprocal(rnu, nu)
                nc.vector.tensor_tensor(cell[b], q, rnu, OP.mult)
                nc.vector.tensor_reduce(rmin[b], cell[b], AX, OP.min)
                sq = smallp.tile([H, N], F32, tag=f"dsq{b}", name=f"dsq{b}")
                nc.scalar.activation(sq, cell[b], AF.Square, bias=bias_z)
                nc.scalar.activation(decay[b], sq, AF.Exp, bias=bias_z, scale=-2.0)
            # compm^{-1} row
            rrow = smallp.tile([1, N], F32, tag="rrow")
            for b in range(2):
                csq = smallp.tile([H, 1], F32, tag=f"csq{b}", name=f"csq{b}")
                nc.scalar.activation(csq, rmin[b], AF.Square, bias=bias_z)
                cm = smallp.tile([H, 1], F32, tag=f"cm{b}", name=f"cm{b}")
                nc.scalar.activation(cm, csq, AF.Exp, bias=bias_z, scale=-2.0)
                rc = smallp.tile([H, 1], F32, tag=f"rc{b}", name=f"rc{b}")
                nc.vector.reciprocal(rc, cm)
                pr = psep.tile([1, H], F32, tag="prow", name="prow")
                nc.tensor.matmul(pr, lhsT=rc, rhs=ident, start=True, stop=True)
                nc.scalar.copy(rrow[:, b * H:(b + 1) * H], pr)
            prb = psep.tile([H, N], F32, tag="prb")
            nc.tensor.matmul(prb, lhsT=ones_row, rhs=rrow, start=True, stop=True)
            sout = smallp.tile([1, N], F32, tag="sout")
            for b in range(2):
                rat = smallp.tile([H, N], F32, tag=f"rat{b}", name=f"rat{b}")
                nc.vector.tensor_tensor(rat, decay[b], prb, OP.mult)
                coef = smallp.tile([H, 1], F32, tag=f"coef{b}", name=f"coef{b}")
                nc.vector.tensor_reduce(coef, rat, AX, OP.min)
                sc = smallp.tile([H, 1], F32, tag=f"sc{b}", name=f"sc{b}")
                nc.vector.tensor_tensor(sc, coef, scol[b], OP.mult)
                pr = psep.tile([1, H], F32, tag="prow", name="prow")
                nc.tensor.matmul(pr, lhsT=sc, rhs=ident, start=True, stop=True)
                nc.scalar.copy(sout[:, b * H:(b + 1) * H], pr)
            nc.sync.dma_start(out=scores_out[:, :], in_=sout)
            iterstack.close()
    nc.finalize()
    return nc


_NC_CACHE = None


def _get_nc():
    global _NC_CACHE
    if _NC_CACHE is None:
        _NC_CACHE = _build_nc()
    return _NC_CACHE


def _host_inputs(seg_masks, cate_scores, feature_map, x, targets, cate_labels):
    seg = np.asarray(seg_masks, np.float32).reshape(N, -1)
    gfull = (np.asarray(feature_map, np.float32) + 10.0)
    xf = np.asarray(x, np.float32)
    tf = np.asarray(targets, np.float32)
    labels = np.asarray(cate_labels)
    a = np.arange(N)
    maskT = ((a[None, :] < a[:, None]) &
             (labels[:, None] == labels[None, :])).astype(np.float32)
    scol = np.asarray(cate_scores, np.float32).reshape(N, 1)
    sh_p1 = np.eye(H, H, -1, dtype=np.float32)
    sh_m1 = np.eye(H, H, 1, dtype=np.float32)
    ident = np.eye(H, dtype=np.float32)
    ones_row = np.ones((1, H), np.float32)

    in_maps = []
    for c in range(NCORES):
        g = np.zeros((3, H, WT), np.float32)
        xp = np.zeros((H, WT), np.float32)
        tp = np.zeros((H, WT), np.float32)
        for i in range(NI):
            inst = c * NI + i
            g[:, :, _strip(i)] = gfull[inst]
            xp[:, _strip(i)] = xf[inst, 0]
            tp[:, _strip(i)] = tf[inst, 0]
        segT = np.empty((KC, N + 1), ml_dtypes.bfloat16)
        segT[:, :N] = seg[:, c * KC:(c + 1) * KC].T.astype(ml_dtypes.bfloat16)
        segT[:, N] = 1
        in_maps.append({
            "g": g, "xp": xp, "tp": tp, "segT": segT, "maskT": maskT,
            "scores_col": scol, "sh_p1": sh_p1, "sh_m1": sh_m1,
            "ident": ident, "ones_row": ones_row,
        })
    return in_maps


def kernel(seg_masks, cate_scores, feature_map, x, targets, cate_labels,
           _trace=False, _trace_kwargs=None):
    in_maps = _host_inputs(seg_masks, cate_scores, feature_map, x, targets,
                           cate_labels)
    nc = _get_nc()
    res = run_bass_kernel_spmd(nc, in_maps, core_ids=list(range(NCORES)),
                               trace=_trace, **(_trace_kwargs or {}))
    outs = res.results
    masks = np.empty((NCORES * NI, 1, H, W), np.float32)
    valid = np.empty(NCORES * NI, np.float32)
    for c in range(NCORES):
        mc = outs[c]["masks"].reshape(H, NI, W).transpose(1, 0, 2)
        masks[c * NI:(c + 1) * NI, 0] = mc
        valid[c * NI:(c + 1) * NI] = outs[c]["valid"].reshape(NI)
    scores = outs[0]["scores"].reshape(N).astype(np.float32)
    if _trace:
        kernel._last_results = res
    return scores, masks, valid


# revision 13
# speedup vs baseline: 1.0659x; 1.0659x over previous
"""Trainium2 Bass kernel for nn_DiscoBoxv2Head (matrix-NMS + mean-field CRF).

Self-contained: hardcodes shapes/sharding for the fixed problem size
  seg_masks (256,128,128) f32, cate_scores (256,) f32,
  feature_map (64,3,128,128) f32, x/targets (64,1,128,128) f32,
  cate_labels (256,) int64.

Sharding over 8 cores:
  - CRF: data-parallel over the 64 instances (8 per core).
  - NMS: contraction (K)-sharded m@m.T -> per-core partial (256,257)
    [inter | row-sums] -> AllReduce(add) -> epilogue redundantly per core.

CRF math (exact reformulation of the reference):
  ret only takes values {0.45, 0.55}, so each mean-field iteration is the
  binary update  m <- targets * (sum_o k_o * shift_o(m) > Ksum/2)  with
  k_o = exp(-2*sum_c(dg_c^2) - s_o/1800) (ALPHA0 drops out), k_center = 1,
  and k_{-o}(p) = k_o(p-delta_o).  Zero padding of g = fm+10 makes k
  underflow to exactly 0 at image borders, so shifts can read garbage-free
  zero pads.  Verified bit-identical masks vs the jax reference.
"""
from contextlib import ExitStack

import numpy as np
import ml_dtypes

import concourse.bass as bass
import concourse.bacc as bacc
import concourse.tile as tile
from concourse import mybir
from concourse.bass_utils import run_bass_kernel_spmd

F32 = mybir.dt.float32
BF16 = mybir.dt.bfloat16
AX = mybir.AxisListType.X
OP = mybir.AluOpType
AF = mybir.ActivationFunctionType

NCORES = 8
NI = 8              # instances per core
H = W = 128
STR = 130           # strip stride (1 pad + 128 + 1 pad)
WT = NI * STR + 2   # 1042 tile width
LO, HI = 1, WT - 1  # elementwise compute window [1, 1041)
KC = 2048           # NMS contraction chunk per core
N = 256             # NMS candidates
BIG = 1.0e30

# strip i occupies cols [STR*i+2, STR*i+130)
def _strip(i):
    return slice(STR * i + 2, STR * i + 2 + W)


def _win(t, fo=0):
    """AP over the compute window with free offset fo."""
    return t[:, LO + fo:HI + fo]


def _build_nc():
    nc = bacc.Bacc("TRN2", target_bir_lowering=False, debug=False, num_devices=8)
    # ---- DRAM I/O ----
    g_in = nc.declare_dram_parameter("g", [3, H, WT], F32, isOutput=False)
    xp_in = nc.declare_dram_parameter("xp", [H, WT], F32, isOutput=False)
    tp_in = nc.declare_dram_parameter("tp", [H, WT], F32, isOutput=False)
    segT_in = nc.declare_dram_parameter("segT", [KC, N + 1], BF16, isOutput=False)
    maskT_in = nc.declare_dram_parameter("maskT", [N, N], F32, isOutput=False)
    scor_in = nc.declare_dram_parameter("scores_col", [N, 1], F32, isOutput=False)
    shp1_in = nc.declare_dram_parameter("sh_p1", [H, H], F32, isOutput=False)
    shm1_in = nc.declare_dram_parameter("sh_m1", [H, H], F32, isOutput=False)
    iden_in = nc.declare_dram_parameter("ident", [H, H], F32, isOutput=False)
    ones_in = nc.declare_dram_parameter("ones_row", [1, H], F32, isOutput=False)

    masks_out = nc.declare_dram_parameter("masks", [H, NI * W], F32, isOutput=True)
    valid_out = nc.declare_dram_parameter("valid", [1, NI], F32, isOutput=True)
    scores_out = nc.declare_dram_parameter("scores", [1, N], F32, isOutput=True)

    cc_in = nc.dram_tensor("cc_in", [2 * H, N + 1], F32)
    cc_out = nc.dram_tensor("cc_out", [2 * H, N + 1], F32)

    # PE chunk windows (with 2-col overlap) covering [0, WT)
    CH = [(0, 262), (260, 522), (520, 782), (780, 1042)]

    with tile.TileContext(nc, num_cores=NCORES) as tc:
        with (
            tc.tile_pool(name="const", bufs=1) as constp,
            tc.tile_pool(name="kf", bufs=1) as kfp,
            tc.tile_pool(name="mst", bufs=1) as mstp,
            tc.tile_pool(name="psum", bufs=1, space="PSUM") as psp,
            tc.tile_pool(name="small", bufs=1) as smallp,
        ):
            # ---------- constants ----------
            ident = constp.tile([H, H], F32, tag="ident")
            nc.sync.dma_start(out=ident, in_=iden_in[:, :])
            ones_row = constp.tile([1, H], F32, tag="ones_row")
            nc.sync.dma_start(out=ones_row, in_=ones_in[:, :])
            sh_p1 = constp.tile([H, H], F32, tag="sh_p1")
            nc.sync.dma_start(out=sh_p1, in_=shp1_in[:, :])
            sh_m1 = constp.tile([H, H], F32, tag="sh_m1")
            nc.sync.dma_start(out=sh_m1, in_=shm1_in[:, :])
            maskT = [constp.tile([H, N], F32, tag=f"maskT{b}", name=f"maskT{b}") for b in range(2)]
            for b in range(2):
                nc.sync.dma_start(out=maskT[b], in_=maskT_in[b * H:(b + 1) * H, :])
            scol = [constp.tile([H, 1], F32, tag=f"scol{b}", name=f"scol{b}") for b in range(2)]
            for b in range(2):
                nc.sync.dma_start(out=scol[b], in_=scor_in[b * H:(b + 1) * H, :])
            tp = constp.tile([H, WT], F32, tag="tp")
            nc.sync.dma_start(out=tp, in_=tp_in[:, :])
            bias_z = constp.tile([H, 1], F32, tag="bias_z")
            nc.vector.memset(bias_z, 0.0)
            bias_s = [constp.tile([H, 1], F32, tag=f"bias_s{j}", name=f"bias_s{j}") for j in (1, 2)]
            nc.vector.memset(bias_s[0], -1.0 / 1800.0)
            nc.vector.memset(bias_s[1], -2.0 / 1800.0)

            # ---------- NMS local GEMM ----------
            with (tc.tile_pool(name="seg", bufs=16) as segp,
                  tc.tile_pool(name="psnms", bufs=1, space="PSUM") as psnms):
                st = []
                for kt in range(16):
                    s = segp.tile([H, N + 1], BF16, tag="st", name=f"st{kt}")
                    nc.sync.dma_start(out=s, in_=segT_in[kt * H:(kt + 1) * H, :])
                    st.append(s)
                pn = [psnms.tile([H, N + 1], F32, tag=f"pn{b}", name=f"pn{b}") for b in range(2)]
                for ib in range(2):
                    for kt in range(16):
                        nc.tensor.matmul(
                            pn[ib], lhsT=st[kt][:, ib * H:(ib + 1) * H],
                            rhs=st[kt][:, :], start=(kt == 0), stop=(kt == 15))
                inter_l = [smallp.tile([H, N + 1], F32, tag=f"interl{b}", name=f"interl{b}") for b in range(2)]
                for b in range(2):
                    nc.scalar.copy(inter_l[b], pn[b])
                    nc.sync.dma_start(out=cc_in[b * H:(b + 1) * H, :], in_=inter_l[b])
            nc.gpsimd.collective_compute(
                "AllReduce", OP.add,
                replica_groups=[list(range(NCORES))],
                ins=[cc_in[:, :].opt()], outs=[cc_out[:, :].opt()])

            # ---------- CRF precompute: kernel fields ----------
            kE = kfp.tile([H, WT], F32, tag="kE")
            kSE = kfp.tile([H, WT], F32, tag="kSE")
            kS = kfp.tile([H, WT], F32, tag="kS")
            kSW = kfp.tile([H, WT], F32, tag="kSW")
            kSEu = kfp.tile([H, WT], F32, tag="kSEu")
            kSu = kfp.tile([H, WT], F32, tag="kSu")
            kSWu = kfp.tile([H, WT], F32, tag="kSWu")
            for t in (kE, kSE, kS, kSW):
                nc.vector.memset(t[:, 0:1], 0.0)
                nc.vector.memset(t[:, WT - 1:WT], 0.0)

            with tc.tile_pool(name="pre", bufs=1) as prep:
                g = []
                for c in range(3):
                    gt = prep.tile([H, WT], F32, tag=f"g{c}", name=f"g{c}")
                    nc.sync.dma_start(out=gt, in_=g_in[c, :, :])
                    g.append(gt)
                # gds[c] = g_c shifted: gds(i,j) = g_c(i+1, j)
                gds = []
                for c in range(3):
                    gd = prep.tile([H, WT], F32, tag=f"gd{c}", name=f"gd{c}")
                    for (a, bnd) in CH:
                        pch = psp.tile([H, 262], F32, tag="chunk", name="pch", bufs=4)
                        nc.tensor.matmul(pch[:, 0:bnd - a], lhsT=sh_p1,
                                         rhs=g[c][:, a:bnd], start=True, stop=True)
                        nc.scalar.copy(gd[:, a:bnd], pch[:, 0:bnd - a])
                    gds.append(gd)

                # field defs: (name, ktile, src fn, free offset, spatial s)
                fields = [
                    ("E", kE, g, 1, 1.0),
                    ("SE", kSE, gds, 1, 2.0),
                    ("S", kS, gds, 0, 1.0),
                    ("SW", kSW, gds, -1, 2.0),
                ]
                eng_d = [nc.vector, nc.gpsimd]
                di = 0
                for fi, (nm, ktile, src, fo, s) in enumerate(fields):
                    sqs = []
                    for c in range(3):
                        d = prep.tile([H, WT], F32, tag="d", name="d", bufs=3)
                        e = eng_d[di % 2]; di += 1
                        e.tensor_tensor(_win(d), _win(src[c], fo), _win(g[c]),
                                        OP.subtract)
                        sq = prep.tile([H, WT], F32, tag="sq", name="sq", bufs=3)
                        nc.scalar.activation(_win(sq), _win(d), AF.Square, bias=bias_z)
                        sqs.append(sq)
                    ss = prep.tile([H, WT], F32, tag="ss", bufs=2)
                    e = eng_d[di % 2]; di += 1
                    e.tensor_tensor(_win(ss), _win(sqs[0]), _win(sqs[1]), OP.add)
                    e = eng_d[di % 2]; di += 1
                    e.tensor_tensor(_win(ss), _win(ss), _win(sqs[2]), OP.add)
                    nc.scalar.activation(_win(ktile), _win(ss), AF.Exp,
                                         bias=bias_s[int(s) - 1], scale=-2.0)

                # opposite-direction fields via PE row-shift up
                for ksrc, kdst in ((kSE, kSEu), (kS, kSu), (kSW, kSWu)):
                    for (a, bnd) in CH:
                        pch = psp.tile([H, 262], F32, tag="chunk", name="pch", bufs=4)
                        nc.tensor.matmul(pch[:, 0:bnd - a], lhsT=sh_m1,
                                         rhs=ksrc[:, a:bnd], start=True, stop=True)
                        nc.scalar.copy(kdst[:, a:bnd], pch[:, 0:bnd - a])

                # Ksum -> threshold tile (with target gate folded in)
                ha = prep.tile([H, WT], F32, tag="ha")
                hb = prep.tile([H, WT], F32, tag="hb")
                hc = prep.tile([H, WT], F32, tag="hc")
                hd = prep.tile([H, WT], F32, tag="hd")
                nc.vector.tensor_tensor(_win(ha), _win(kE), _win(kE, -1), OP.add)
                nc.gpsimd.tensor_tensor(_win(hb), _win(kSE), _win(kSEu, -1), OP.add)
                nc.vector.tensor_tensor(_win(hc), _win(kS), _win(kSu), OP.add)
                nc.gpsimd.tensor_tensor(_win(hd), _win(kSW), _win(kSWu, 1), OP.add)
                nc.vector.tensor_tensor(_win(ha), _win(ha), _win(hb), OP.add)
                nc.gpsimd.tensor_tensor(_win(hc), _win(hc), _win(hd), OP.add)
                nc.vector.tensor_tensor(_win(ha), _win(ha), _win(hc), OP.add)
                kthr = mstp.tile([H, WT], F32, tag="kthr")
                nc.gpsimd.memset(kthr, BIG)
                kpre = prep.tile([H, WT], F32, tag="kpre")
                nc.vector.tensor_scalar(_win(kpre), _win(ha), 1.0, 0.5,
                                        OP.add, OP.mult)
                # gate = (1-tp)*BIG in {0, BIG}; kthr = kpre + gate is exact
                # where tp==1 and a huge threshold (mask stays 0) where tp==0
                gate = prep.tile([H, WT], F32, tag="gate")
                nc.vector.tensor_scalar(_win(gate), _win(tp), -BIG, BIG,
                                        OP.mult, OP.add)
                nc.vector.tensor_tensor(_win(kthr), _win(kpre), _win(gate), OP.add)

                # m0 = (x*t > 0.5)
                mA = mstp.tile([H, WT], F32, tag="mA")
                mB = mstp.tile([H, WT], F32, tag="mB")
                nc.gpsimd.memset(mA, 0.0)
                nc.gpsimd.memset(mB, 0.0)
                xt = prep.tile([H, WT], F32, tag="xt")
                xp = prep.tile([H, WT], F32, tag="xp")
                nc.sync.dma_start(out=xp, in_=xp_in[:, :])
                nc.vector.tensor_tensor(_win(xt), _win(xp), _win(tp), OP.mult)
                nc.vector.tensor_scalar(_win(mA), _win(xt), 0.5, None, OP.is_gt)

            # ---------- CRF iterations ----------
            iterstack = ExitStack()
            prodp = iterstack.enter_context(tc.tile_pool(name="prod", bufs=8))
            accp = iterstack.enter_context(tc.tile_pool(name="accs", bufs=2))
            mshp = iterstack.enter_context(tc.tile_pool(name="mshift", bufs=2))
            mtiles = [mA, mB]
            for it in range(10):
                mc = mtiles[it % 2]
                mn = mtiles[(it + 1) % 2]
                mdn = mshp.tile([H, WT], F32, tag="mdn")
                mup = mshp.tile([H, WT], F32, tag="mup")
                for sh, dst in ((sh_p1, mdn), (sh_m1, mup)):
                    for (a, bnd) in CH:
                        pch = psp.tile([H, 262], F32, tag="chunk", name="pch", bufs=4)
                        nc.tensor.matmul(pch[:, 0:bnd - a], lhsT=sh,
                                         rhs=mc[:, a:bnd], start=True, stop=True)
                        nc.scalar.copy(dst[:, a:bnd], pch[:, 0:bnd - a])
                P = [prodp.tile([H, WT], F32, tag="P", name=f"P{j}") for j in range(8)]
                nc.vector.tensor_tensor(_win(P[0]), _win(kE), _win(mc, 1), OP.mult)
                nc.vector.tensor_tensor(_win(P[1]), _win(kE, -1), _win(mc, -1), OP.mult)
                nc.vector.tensor_tensor(_win(P[2]), _win(kSE), _win(mdn, 1), OP.mult)
                nc.vector.tensor_tensor(_win(P[3]), _win(kS), _win(mdn), OP.mult)
                nc.gpsimd.tensor_tensor(_win(P[4]), _win(kSW), _win(mdn, -1), OP.mult)
                nc.gpsimd.tensor_tensor(_win(P[5]), _win(kSEu, -1), _win(mup, -1), OP.mult)
                nc.gpsimd.tensor_tensor(_win(P[6]), _win(kSu), _win(mup), OP.mult)
                nc.gpsimd.tensor_tensor(_win(P[7]), _win(kSWu, 1), _win(mup, 1), OP.mult)
                A = accp.tile([H, WT], F32, tag="A")
                B = accp.tile([H, WT], F32, tag="B")
                C = accp.tile([H, WT], F32, tag="C")
                D = accp.tile([H, WT], F32, tag="D")
                nc.vector.tensor_tensor(_win(A), _win(P[0]), _win(P[1]), OP.add)
                nc.vector.tensor_tensor(_win(B), _win(P[2]), _win(P[3]), OP.add)
                nc.gpsimd.tensor_tensor(_win(C), _win(P[4]), _win(P[5]), OP.add)
                nc.gpsimd.tensor_tensor(_win(D), _win(P[6]), _win(P[7]), OP.add)
                nc.vector.tensor_tensor(_win(A), _win(A), _win(B), OP.add)
                nc.vector.tensor_tensor(_win(C), _win(C), _win(D), OP.add)
                nc.vector.tensor_tensor(_win(A), _win(A), _win(mc), OP.add)
                nc.vector.tensor_tensor(_win(A), _win(A), _win(C), OP.add)
                nc.vector.tensor_tensor(_win(mn), _win(A), _win(kthr), OP.is_gt)
            psep = iterstack.enter_context(
                tc.tile_pool(name="psep", bufs=1, space="PSUM"))
            mfin = mtiles[0]

            # ---------- outputs: masks + valid ----------
            for i in range(NI):
                nc.sync.dma_start(out=masks_out[:, i * W:(i + 1) * W],
                                  in_=mfin[:, _strip(i)])
            rs = smallp.tile([H, NI], F32, tag="rs")
            mv = bass.AP(tensor=mfin.tensor, offset=mfin.offset + 2,
                         ap=[mfin.ap[0], [STR, NI], [1, W]])
            nc.vector.tensor_reduce(rs, mv, AX, OP.add)
            ones_col = smallp.tile([H, 1], F32, tag="ones_col")
            nc.vector.memset(ones_col, 1.0)
            pcnt = psep.tile([1, NI], F32, tag="pcnt")
            nc.tensor.matmul(pcnt, lhsT=ones_col, rhs=rs, start=True, stop=True)
            va = smallp.tile([1, NI], F32, tag="va")
            vb = smallp.tile([1, NI], F32, tag="vb")
            nc.vector.tensor_scalar(va, pcnt, float(H * W * 0.05), None, OP.is_ge)
            nc.vector.tensor_scalar(vb, pcnt, float(H * W * 0.95), None, OP.is_le)
            nc.vector.tensor_tensor(va, va, vb, OP.mult)
            nc.sync.dma_start(out=valid_out[:, :], in_=va)

            # ---------- NMS epilogue ----------
            inter = [constp.tile([H, N + 1], F32, tag=f"inter{b}", name=f"inter{b}") for b in range(2)]
            for b in range(2):
                nc.scalar.dma_start(out=inter[b], in_=cc_out[b * H:(b + 1) * H, :])
            # layout [a(part)=j, b(free)=i]; cell(a,b) = -decay_iou[b,a]
            srow = smallp.tile([1, N], F32, tag="srow")
            for b in range(2):
                pr = psep.tile([1, H], F32, tag="prow", name="prow")
                nc.tensor.matmul(pr, lhsT=inter[b][:, N:N + 1], rhs=ident,
                                 start=True, stop=True)
                nc.scalar.copy(srow[:, b * H:(b + 1) * H], pr)
            psb = psep.tile([H, N], F32, tag="psb")
            nc.tensor.matmul(psb, lhsT=ones_row, rhs=srow, start=True, stop=True)
            cell = [smallp.tile([H, N], F32, tag=f"cell{b}", name=f"cell{b}") for b in range(2)]
            rmin = [smallp.tile([H, 1], F32, tag=f"rmin{b}", name=f"rmin{b}") for b in range(2)]
            decay = [smallp.tile([H, N], F32, tag=f"decay{b}", name=f"decay{b}") for b in range(2)]
            for b in range(2):
                nu = smallp.tile([H, N], F32, tag=f"nu{b}", name=f"nu{b}")
                # nu = (inter - s_a) - s_b  == -union
                nc.vector.scalar_tensor_tensor(nu, inter[b][:, 0:N],
                                               inter[b][:, N:N + 1], psb,
                                               OP.subtract, OP.subtract)
                q = smallp.tile([H, N], F32, tag=f"q{b}", name=f"q{b}")
                nc.vector.tensor_tensor(q, inter[b][:, 0:N], maskT[b], OP.mult)
                rnu = smallp.tile([H, N], F32, tag=f"rnu{b}", name=f"rnu{b}")
                nc.vector.reciprocal(rnu, nu)
                nc.vector.tensor_tensor(cell[b], q, rnu, OP.mult)
                nc.vector.tensor_reduce(rmin[b], cell[b], AX, OP.min)
                sq = smallp.tile([H, N], F32, tag=f"dsq{b}", name=f"dsq{b}")
                nc.scalar.activation(sq, cell[b], AF.Square, bias=bias_z)
                nc.scalar.activation(decay[b], sq, AF.Exp, bias=bias_z, scale=-2.0)
            # compm^{-1} row
            rrow = smallp.tile([1, N], F32, tag="rrow")
            for b in range(2):
                csq = smallp.tile([H, 1], F32, tag=f"csq{b}", name=f"csq{b}")
                nc.scalar.activation(csq, rmin[b], AF.Square, bias=bias_z)
                cm = smallp.tile([H, 1], F32, tag=f"cm{b}", name=f"cm{b}")
                nc.scalar.activation(cm, csq, AF.Exp, bias=bias_z, scale=-2.0)
                rc = smallp.tile([H, 1], F32, tag=f"rc{b}", name=f"rc{b}")
                nc.vector.reciprocal(rc, cm)
                pr = psep.tile([1, H], F32, tag="prow", name="prow")
                nc.tensor.matmul(pr, lhsT=rc, rhs=ident, start=True, stop=True)
                nc.scalar.copy(rrow[:, b * H:(b + 1) * H], pr)
            prb = psep.tile([H, N], F32, tag="prb")
            nc.tensor.matmul(prb, lhsT=ones_row, rhs=rrow, start=True, stop=True)
            sout = smallp.tile([1, N], F32, tag="sout")
            for b in range(2):
                rat = smallp.tile([H, N], F32, tag=f"rat{b}", name=f"rat{b}")
                nc.vector.tensor_tensor(rat, decay[b], prb, OP.mult)
                coef = smallp.tile([H, 1], F32, tag=f"coef{b}", name=f"coef{b}")
                nc.vector.tensor_reduce(coef, rat, AX, OP.min)
                sc = smallp.tile([H, 1], F32, tag=f"sc{b}", name=f"sc{b}")
                nc.vector.tensor_tensor(sc, coef, scol[b], OP.mult)
                pr = psep.tile([1, H], F32, tag="prow", name="prow")
                nc.tensor.matmul(pr, lhsT=sc, rhs=ident, start=True, stop=True)
                nc.scalar.copy(sout[:, b * H:(b + 1) * H], pr)
            nc.sync.dma_start(out=scores_out[:, :], in_=sout)
            iterstack.close()
    nc.finalize()
    return nc


_NC_CACHE = None


def _get_nc():
    global _NC_CACHE
    if _NC_CACHE is None:
        _NC_CACHE = _build_nc()
    return _NC_CACHE


def _host_inputs(seg_masks, cate_scores, feature_map, x, targets, cate_labels):
    seg = np.asarray(seg_masks, np.float32).reshape(N, -1)
    gfull = (np.asarray(feature_map, np.float32) + 10.0)
    xf = np.asarray(x, np.float32)
    tf = np.asarray(targets, np.float32)
    labels = np.asarray(cate_labels)
    a = np.arange(N)
    maskT = ((a[None, :] < a[:, None]) &
             (labels[:, None] == labels[None, :])).astype(np.float32)
    scol = np.asarray(cate_scores, np.float32).reshape(N, 1)
    sh_p1 = np.eye(H, H, -1, dtype=np.float32)
    sh_m1 = np.eye(H, H, 1, dtype=np.float32)
    ident = np.eye(H, dtype=np.float32)
    ones_row = np.ones((1, H), np.float32)

    in_maps = []
    for c in range(NCORES):
        g = np.zeros((3, H, WT), np.float32)
        xp = np.zeros((H, WT), np.float32)
        tp = np.zeros((H, WT), np.float32)
        for i in range(NI):
            inst = c * NI + i
            g[:, :, _strip(i)] = gfull[inst]
            xp[:, _strip(i)] = xf[inst, 0]
            tp[:, _strip(i)] = tf[inst, 0]
        segT = np.empty((KC, N + 1), ml_dtypes.bfloat16)
        segT[:, :N] = seg[:, c * KC:(c + 1) * KC].T.astype(ml_dtypes.bfloat16)
        segT[:, N] = 1
        in_maps.append({
            "g": g, "xp": xp, "tp": tp, "segT": segT, "maskT": maskT,
            "scores_col": scol, "sh_p1": sh_p1, "sh_m1": sh_m1,
            "ident": ident, "ones_row": ones_row,
        })
    return in_maps


def kernel(seg_masks, cate_scores, feature_map, x, targets, cate_labels,
           _trace=False, _trace_kwargs=None):
    in_maps = _host_inputs(seg_masks, cate_scores, feature_map, x, targets,
                           cate_labels)
    nc = _get_nc()
    res = run_bass_kernel_spmd(nc, in_maps, core_ids=list(range(NCORES)),
                               trace=_trace, **(_trace_kwargs or {}))
    outs = res.results
    masks = np.empty((NCORES * NI, 1, H, W), np.float32)
    valid = np.empty(NCORES * NI, np.float32)
    for c in range(NCORES):
        mc = outs[c]["masks"].reshape(H, NI, W).transpose(1, 0, 2)
        masks[c * NI:(c + 1) * NI, 0] = mc
        valid[c * NI:(c + 1) * NI] = outs[c]["valid"].reshape(NI)
    scores = outs[0]["scores"].reshape(N).astype(np.float32)
    if _trace:
        kernel._last_results = res
    return scores, masks, valid


# revision 14
# speedup vs baseline: 1.1890x; 1.1155x over previous
"""Trainium2 Bass kernel for nn_DiscoBoxv2Head (matrix-NMS + mean-field CRF).

Self-contained: hardcodes shapes/sharding for the fixed problem size
  seg_masks (256,128,128) f32, cate_scores (256,) f32,
  feature_map (64,3,128,128) f32, x/targets (64,1,128,128) f32,
  cate_labels (256,) int64.

Sharding over 8 cores:
  - CRF: data-parallel over the 64 instances (8 per core).
  - NMS: contraction (K)-sharded m@m.T -> per-core partial (256,257)
    [inter | row-sums] -> AllReduce(add) -> epilogue redundantly per core.

CRF math (exact reformulation of the reference):
  ret only takes values {0.45, 0.55}, so each mean-field iteration is the
  binary update  m <- targets * (sum_o k_o * shift_o(m) > Ksum/2)  with
  k_o = exp(-2*sum_c(dg_c^2) - s_o/1800) (ALPHA0 drops out), k_center = 1,
  and k_{-o}(p) = k_o(p-delta_o).  Zero padding of g = fm+10 makes k
  underflow to exactly 0 at image borders, so shifts can read garbage-free
  zero pads.  Verified bit-identical masks vs the jax reference.
"""
from contextlib import ExitStack

import numpy as np
import ml_dtypes

import concourse.bass as bass
import concourse.bacc as bacc
import concourse.tile as tile
from concourse import mybir
from concourse.bass_utils import run_bass_kernel_spmd

F32 = mybir.dt.float32
BF16 = mybir.dt.bfloat16
AX = mybir.AxisListType.X
OP = mybir.AluOpType
AF = mybir.ActivationFunctionType

NCORES = 8
NI = 8              # instances per core
H = W = 128
STR = 130           # strip stride (1 pad + 128 + 1 pad)
WT = NI * STR + 2   # 1042 tile width
LO, HI = 1, WT - 1  # elementwise compute window [1, 1041)
KC = 2048           # NMS contraction chunk per core
N = 256             # NMS candidates
BIG = 1.0e30

# strip i occupies cols [STR*i+2, STR*i+130)
def _strip(i):
    return slice(STR * i + 2, STR * i + 2 + W)


def _win(t, fo=0):
    """AP over the compute window with free offset fo."""
    return t[:, LO + fo:HI + fo]


def _build_nc():
    nc = bacc.Bacc("TRN2", target_bir_lowering=False, debug=False, num_devices=8)
    # ---- DRAM I/O ----
    g_in = nc.declare_dram_parameter("g", [3, H, WT], F32, isOutput=False)
    xp_in = nc.declare_dram_parameter("xp", [H, WT], F32, isOutput=False)
    tp_in = nc.declare_dram_parameter("tp", [H, WT], F32, isOutput=False)
    segT_in = nc.declare_dram_parameter("segT", [KC, N + 1], BF16, isOutput=False)
    maskT_in = nc.declare_dram_parameter("maskT", [N, N], F32, isOutput=False)
    scor_in = nc.declare_dram_parameter("scores_col", [N, 1], F32, isOutput=False)
    shp1_in = nc.declare_dram_parameter("sh_p1", [H, H], F32, isOutput=False)
    shm1_in = nc.declare_dram_parameter("sh_m1", [H, H], F32, isOutput=False)
    iden_in = nc.declare_dram_parameter("ident", [H, H], F32, isOutput=False)
    ones_in = nc.declare_dram_parameter("ones_row", [1, H], F32, isOutput=False)

    masks_out = nc.declare_dram_parameter("masks", [H, NI * W], F32, isOutput=True)
    valid_out = nc.declare_dram_parameter("valid", [1, NI], F32, isOutput=True)
    scores_out = nc.declare_dram_parameter("scores", [1, N], F32, isOutput=True)

    cc_in = nc.dram_tensor("cc_in", [2 * H, N + 1], F32)
    cc_out = nc.dram_tensor("cc_out", [2 * H, N + 1], F32)

    # PE chunk windows (with 2-col overlap) covering [0, WT)
    CH = [(0, 262), (260, 522), (520, 782), (780, 1042)]

    with tile.TileContext(nc, num_cores=NCORES) as tc:
        with (
            tc.tile_pool(name="const", bufs=1) as constp,
            tc.tile_pool(name="kf", bufs=1) as kfp,
            tc.tile_pool(name="mst", bufs=1) as mstp,
            tc.tile_pool(name="psum", bufs=1, space="PSUM") as psp,
            tc.tile_pool(name="small", bufs=1) as smallp,
        ):
            # ---------- constants ----------
            ident = constp.tile([H, H], F32, tag="ident")
            nc.sync.dma_start(out=ident, in_=iden_in[:, :])
            ones_row = constp.tile([1, H], F32, tag="ones_row")
            nc.sync.dma_start(out=ones_row, in_=ones_in[:, :])
            sh_p1 = constp.tile([H, H], F32, tag="sh_p1")
            nc.sync.dma_start(out=sh_p1, in_=shp1_in[:, :])
            sh_m1 = constp.tile([H, H], F32, tag="sh_m1")
            nc.sync.dma_start(out=sh_m1, in_=shm1_in[:, :])
            maskT = [constp.tile([H, N], F32, tag=f"maskT{b}", name=f"maskT{b}") for b in range(2)]
            for b in range(2):
                nc.sync.dma_start(out=maskT[b], in_=maskT_in[b * H:(b + 1) * H, :])
            scol = [constp.tile([H, 1], F32, tag=f"scol{b}", name=f"scol{b}") for b in range(2)]
            for b in range(2):
                nc.sync.dma_start(out=scol[b], in_=scor_in[b * H:(b + 1) * H, :])
            tp = constp.tile([H, WT], F32, tag="tp")
            nc.sync.dma_start(out=tp, in_=tp_in[:, :])
            bias_z = constp.tile([H, 1], F32, tag="bias_z")
            nc.vector.memset(bias_z, 0.0)
            bias_s = [constp.tile([H, 1], F32, tag=f"bias_s{j}", name=f"bias_s{j}") for j in (1, 2)]
            nc.vector.memset(bias_s[0], -1.0 / 1800.0)
            nc.vector.memset(bias_s[1], -2.0 / 1800.0)

            # ---------- NMS local GEMM ----------
            with (tc.tile_pool(name="seg", bufs=16) as segp,
                  tc.tile_pool(name="psnms", bufs=1, space="PSUM") as psnms):
                st = []
                for kt in range(16):
                    s = segp.tile([H, N + 1], BF16, tag="st", name=f"st{kt}")
                    nc.sync.dma_start(out=s, in_=segT_in[kt * H:(kt + 1) * H, :])
                    st.append(s)
                pn = [psnms.tile([H, N + 1], F32, tag=f"pn{b}", name=f"pn{b}") for b in range(2)]
                for ib in range(2):
                    for kt in range(16):
                        nc.tensor.matmul(
                            pn[ib], lhsT=st[kt][:, ib * H:(ib + 1) * H],
                            rhs=st[kt][:, :], start=(kt == 0), stop=(kt == 15))
                inter_l = [smallp.tile([H, N + 1], F32, tag=f"interl{b}", name=f"interl{b}") for b in range(2)]
                for b in range(2):
                    nc.scalar.copy(inter_l[b], pn[b])
                    nc.sync.dma_start(out=cc_in[b * H:(b + 1) * H, :], in_=inter_l[b])
            nc.gpsimd.collective_compute(
                "AllReduce", OP.add,
                replica_groups=[list(range(NCORES))],
                ins=[cc_in[:, :].opt()], outs=[cc_out[:, :].opt()])

            # ---------- CRF precompute: kernel fields ----------
            kE = kfp.tile([H, WT], F32, tag="kE")
            kSE = kfp.tile([H, WT], F32, tag="kSE")
            kS = kfp.tile([H, WT], F32, tag="kS")
            kSW = kfp.tile([H, WT], F32, tag="kSW")
            kSEu = kfp.tile([H, WT], F32, tag="kSEu")
            kSu = kfp.tile([H, WT], F32, tag="kSu")
            kSWu = kfp.tile([H, WT], F32, tag="kSWu")
            for t in (kE, kSE, kS, kSW):
                nc.vector.memset(t[:, 0:1], 0.0)
                nc.vector.memset(t[:, WT - 1:WT], 0.0)

            with tc.tile_pool(name="pre", bufs=1) as prep:
                g = []
                for c in range(3):
                    gt = prep.tile([H, WT], F32, tag=f"g{c}", name=f"g{c}")
                    nc.sync.dma_start(out=gt, in_=g_in[c, :, :])
                    g.append(gt)
                # gds[c] = g_c shifted: gds(i,j) = g_c(i+1, j)
                gds = []
                for c in range(3):
                    gd = prep.tile([H, WT], F32, tag=f"gd{c}", name=f"gd{c}")
                    for (a, bnd) in CH:
                        pch = psp.tile([H, 262], F32, tag="chunk", name="pch", bufs=4)
                        nc.tensor.matmul(pch[:, 0:bnd - a], lhsT=sh_p1,
                                         rhs=g[c][:, a:bnd], start=True, stop=True)
                        nc.scalar.copy(gd[:, a:bnd], pch[:, 0:bnd - a])
                    gds.append(gd)

                # field defs: (name, ktile, src fn, free offset, spatial s)
                fields = [
                    ("E", kE, g, 1, 1.0),
                    ("SE", kSE, gds, 1, 2.0),
                    ("S", kS, gds, 0, 1.0),
                    ("SW", kSW, gds, -1, 2.0),
                ]
                eng_d = [nc.vector, nc.vector]
                di = 0
                for fi, (nm, ktile, src, fo, s) in enumerate(fields):
                    sqs = []
                    for c in range(3):
                        d = prep.tile([H, WT], F32, tag="d", name="d", bufs=3)
                        e = eng_d[di % 2]; di += 1
                        e.tensor_tensor(_win(d), _win(src[c], fo), _win(g[c]),
                                        OP.subtract)
                        sq = prep.tile([H, WT], F32, tag="sq", name="sq", bufs=3)
                        nc.scalar.activation(_win(sq), _win(d), AF.Square, bias=bias_z)
                        sqs.append(sq)
                    ss = prep.tile([H, WT], F32, tag="ss", bufs=2)
                    e = eng_d[di % 2]; di += 1
                    e.tensor_tensor(_win(ss), _win(sqs[0]), _win(sqs[1]), OP.add)
                    e = eng_d[di % 2]; di += 1
                    e.tensor_tensor(_win(ss), _win(ss), _win(sqs[2]), OP.add)
                    nc.scalar.activation(_win(ktile), _win(ss), AF.Exp,
                                         bias=bias_s[int(s) - 1], scale=-2.0)

                # opposite-direction fields via PE row-shift up
                for ksrc, kdst in ((kSE, kSEu), (kS, kSu), (kSW, kSWu)):
                    for (a, bnd) in CH:
                        pch = psp.tile([H, 262], F32, tag="chunk", name="pch", bufs=4)
                        nc.tensor.matmul(pch[:, 0:bnd - a], lhsT=sh_m1,
                                         rhs=ksrc[:, a:bnd], start=True, stop=True)
                        nc.scalar.copy(kdst[:, a:bnd], pch[:, 0:bnd - a])

                # Ksum -> threshold tile (with target gate folded in)
                ha = prep.tile([H, WT], F32, tag="ha")
                hb = prep.tile([H, WT], F32, tag="hb")
                hc = prep.tile([H, WT], F32, tag="hc")
                hd = prep.tile([H, WT], F32, tag="hd")
                nc.vector.tensor_tensor(_win(ha), _win(kE), _win(kE, -1), OP.add)
                nc.vector.tensor_tensor(_win(hb), _win(kSE), _win(kSEu, -1), OP.add)
                nc.vector.tensor_tensor(_win(hc), _win(kS), _win(kSu), OP.add)
                nc.vector.tensor_tensor(_win(hd), _win(kSW), _win(kSWu, 1), OP.add)
                nc.vector.tensor_tensor(_win(ha), _win(ha), _win(hb), OP.add)
                nc.vector.tensor_tensor(_win(hc), _win(hc), _win(hd), OP.add)
                nc.vector.tensor_tensor(_win(ha), _win(ha), _win(hc), OP.add)
                kthr = mstp.tile([H, WT], F32, tag="kthr")
                nc.vector.memset(kthr, BIG)
                kpre = prep.tile([H, WT], F32, tag="kpre")
                nc.vector.tensor_scalar(_win(kpre), _win(ha), 1.0, 0.5,
                                        OP.add, OP.mult)
                # gate = (1-tp)*BIG in {0, BIG}; kthr = kpre + gate is exact
                # where tp==1 and a huge threshold (mask stays 0) where tp==0
                gate = prep.tile([H, WT], F32, tag="gate")
                nc.vector.tensor_scalar(_win(gate), _win(tp), -BIG, BIG,
                                        OP.mult, OP.add)
                nc.vector.tensor_tensor(_win(kthr), _win(kpre), _win(gate), OP.add)

                # m0 = (x*t > 0.5)
                mA = mstp.tile([H, WT], F32, tag="mA")
                mB = mstp.tile([H, WT], F32, tag="mB")
                nc.vector.memset(mA, 0.0)
                nc.vector.memset(mB, 0.0)
                xt = prep.tile([H, WT], F32, tag="xt")
                xp = prep.tile([H, WT], F32, tag="xp")
                nc.sync.dma_start(out=xp, in_=xp_in[:, :])
                nc.vector.tensor_tensor(_win(xt), _win(xp), _win(tp), OP.mult)
                nc.vector.tensor_scalar(_win(mA), _win(xt), 0.5, None, OP.is_gt)

            # ---------- CRF iterations ----------
            iterstack = ExitStack()
            prodp = iterstack.enter_context(tc.tile_pool(name="prod", bufs=8))
            accp = iterstack.enter_context(tc.tile_pool(name="accs", bufs=2))
            mshp = iterstack.enter_context(tc.tile_pool(name="mshift", bufs=2))
            mtiles = [mA, mB]
            for it in range(10):
                mc = mtiles[it % 2]
                mn = mtiles[(it + 1) % 2]
                mdn = mshp.tile([H, WT], F32, tag="mdn")
                mup = mshp.tile([H, WT], F32, tag="mup")
                for sh, dst in ((sh_p1, mdn), (sh_m1, mup)):
                    for (a, bnd) in CH:
                        pch = psp.tile([H, 262], F32, tag="chunk", name="pch", bufs=4)
                        nc.tensor.matmul(pch[:, 0:bnd - a], lhsT=sh,
                                         rhs=mc[:, a:bnd], start=True, stop=True)
                        nc.scalar.copy(dst[:, a:bnd], pch[:, 0:bnd - a])
                P = [prodp.tile([H, WT], F32, tag="P", name=f"P{j}") for j in range(8)]
                nc.vector.tensor_tensor(_win(P[0]), _win(kE), _win(mc, 1), OP.mult)
                nc.vector.tensor_tensor(_win(P[1]), _win(kE, -1), _win(mc, -1), OP.mult)
                nc.vector.tensor_tensor(_win(P[2]), _win(kSE), _win(mdn, 1), OP.mult)
                nc.vector.tensor_tensor(_win(P[3]), _win(kS), _win(mdn), OP.mult)
                nc.gpsimd.tensor_tensor(_win(P[4]), _win(kSW), _win(mdn, -1), OP.mult)
                nc.gpsimd.tensor_tensor(_win(P[5]), _win(kSEu, -1), _win(mup, -1), OP.mult)
                nc.gpsimd.tensor_tensor(_win(P[6]), _win(kSu), _win(mup), OP.mult)
                nc.gpsimd.tensor_tensor(_win(P[7]), _win(kSWu, 1), _win(mup, 1), OP.mult)
                A = accp.tile([H, WT], F32, tag="A")
                B = accp.tile([H, WT], F32, tag="B")
                C = accp.tile([H, WT], F32, tag="C")
                D = accp.tile([H, WT], F32, tag="D")
                nc.vector.tensor_tensor(_win(A), _win(P[0]), _win(P[1]), OP.add)
                nc.vector.tensor_tensor(_win(B), _win(P[2]), _win(P[3]), OP.add)
                nc.vector.tensor_tensor(_win(C), _win(P[4]), _win(P[5]), OP.add)
                nc.vector.tensor_tensor(_win(D), _win(P[6]), _win(P[7]), OP.add)
                nc.vector.tensor_tensor(_win(A), _win(A), _win(B), OP.add)
                nc.vector.tensor_tensor(_win(C), _win(C), _win(D), OP.add)
                nc.vector.tensor_tensor(_win(A), _win(A), _win(mc), OP.add)
                nc.vector.tensor_tensor(_win(A), _win(A), _win(C), OP.add)
                nc.vector.tensor_tensor(_win(mn), _win(A), _win(kthr), OP.is_gt)
            psep = iterstack.enter_context(
                tc.tile_pool(name="psep", bufs=1, space="PSUM"))
            mfin = mtiles[0]

            # ---------- outputs: masks + valid ----------
            for i in range(NI):
                nc.sync.dma_start(out=masks_out[:, i * W:(i + 1) * W],
                                  in_=mfin[:, _strip(i)])
            rs = smallp.tile([H, NI], F32, tag="rs")
            mv = bass.AP(tensor=mfin.tensor, offset=mfin.offset + 2,
                         ap=[mfin.ap[0], [STR, NI], [1, W]])
            nc.vector.tensor_reduce(rs, mv, AX, OP.add)
            ones_col = smallp.tile([H, 1], F32, tag="ones_col")
            nc.vector.memset(ones_col, 1.0)
            pcnt = psep.tile([1, NI], F32, tag="pcnt")
            nc.tensor.matmul(pcnt, lhsT=ones_col, rhs=rs, start=True, stop=True)
            va = smallp.tile([1, NI], F32, tag="va")
            vb = smallp.tile([1, NI], F32, tag="vb")
            nc.vector.tensor_scalar(va, pcnt, float(H * W * 0.05), None, OP.is_ge)
            nc.vector.tensor_scalar(vb, pcnt, float(H * W * 0.95), None, OP.is_le)
            nc.vector.tensor_tensor(va, va, vb, OP.mult)
            nc.sync.dma_start(out=valid_out[:, :], in_=va)

            # ---------- NMS epilogue ----------
            inter = [constp.tile([H, N + 1], F32, tag=f"inter{b}", name=f"inter{b}") for b in range(2)]
            for b in range(2):
                nc.scalar.dma_start(out=inter[b], in_=cc_out[b * H:(b + 1) * H, :])
            # layout [a(part)=j, b(free)=i]; cell(a,b) = -decay_iou[b,a]
            srow = smallp.tile([1, N], F32, tag="srow")
            for b in range(2):
                pr = psep.tile([1, H], F32, tag="prow", name="prow")
                nc.tensor.matmul(pr, lhsT=inter[b][:, N:N + 1], rhs=ident,
                                 start=True, stop=True)
                nc.scalar.copy(srow[:, b * H:(b + 1) * H], pr)
            psb = psep.tile([H, N], F32, tag="psb")
            nc.tensor.matmul(psb, lhsT=ones_row, rhs=srow, start=True, stop=True)
            cell = [smallp.tile([H, N], F32, tag=f"cell{b}", name=f"cell{b}") for b in range(2)]
            rmin = [smallp.tile([H, 1], F32, tag=f"rmin{b}", name=f"rmin{b}") for b in range(2)]
            decay = [smallp.tile([H, N], F32, tag=f"decay{b}", name=f"decay{b}") for b in range(2)]
            for b in range(2):
                nu = smallp.tile([H, N], F32, tag=f"nu{b}", name=f"nu{b}")
                # nu = (inter - s_a) - s_b  == -union
                nc.vector.scalar_tensor_tensor(nu, inter[b][:, 0:N],
                                               inter[b][:, N:N + 1], psb,
                                               OP.subtract, OP.subtract)
                q = smallp.tile([H, N], F32, tag=f"q{b}", name=f"q{b}")
                nc.vector.tensor_tensor(q, inter[b][:, 0:N], maskT[b], OP.mult)
                rnu = smallp.tile([H, N], F32, tag=f"rnu{b}", name=f"rnu{b}")
                nc.vector.reciprocal(rnu, nu)
                nc.vector.tensor_tensor(cell[b], q, rnu, OP.mult)
                nc.vector.tensor_reduce(rmin[b], cell[b], AX, OP.min)
                sq = smallp.tile([H, N], F32, tag=f"dsq{b}", name=f"dsq{b}")
                nc.scalar.activation(sq, cell[b], AF.Square, bias=bias_z)
                nc.scalar.activation(decay[b], sq, AF.Exp, bias=bias_z, scale=-2.0)
            # compm^{-1} row
            rrow = smallp.tile([1, N], F32, tag="rrow")
            for b in range(2):
                csq = smallp.tile([H, 1], F32, tag=f"csq{b}", name=f"csq{b}")
                nc.scalar.activation(csq, rmin[b], AF.Square, bias=bias_z)
                cm = smallp.tile([H, 1], F32, tag=f"cm{b}", name=f"cm{b}")
                nc.scalar.activation(cm, csq, AF.Exp, bias=bias_z, scale=-2.0)
                rc = smallp.tile([H, 1], F32, tag=f"rc{b}", name=f"rc{b}")
                nc.vector.reciprocal(rc, cm)
                pr = psep.tile([1, H], F32, tag="prow", name="prow")
                nc.tensor.matmul(pr, lhsT=rc, rhs=ident, start=True, stop=True)
                nc.scalar.copy(rrow[:, b * H:(b + 1) * H], pr)
            prb = psep.tile([H, N], F32, tag="prb")
            nc.tensor.matmul(prb, lhsT=ones_row, rhs=rrow, start=True, stop=True)
            sout = smallp.tile([1, N], F32, tag="sout")
            for b in range(2):
                rat = smallp.tile([H, N], F32, tag=f"rat{b}", name=f"rat{b}")
                nc.vector.tensor_tensor(rat, decay[b], prb, OP.mult)
                coef = smallp.tile([H, 1], F32, tag=f"coef{b}", name=f"coef{b}")
                nc.vector.tensor_reduce(coef, rat, AX, OP.min)
                sc = smallp.tile([H, 1], F32, tag=f"sc{b}", name=f"sc{b}")
                nc.vector.tensor_tensor(sc, coef, scol[b], OP.mult)
                pr = psep.tile([1, H], F32, tag="prow", name="prow")
                nc.tensor.matmul(pr, lhsT=sc, rhs=ident, start=True, stop=True)
                nc.scalar.copy(sout[:, b * H:(b + 1) * H], pr)
            nc.sync.dma_start(out=scores_out[:, :], in_=sout)
            iterstack.close()
    nc.finalize()
    return nc


_NC_CACHE = None


def _get_nc():
    global _NC_CACHE
    if _NC_CACHE is None:
        _NC_CACHE = _build_nc()
    return _NC_CACHE


def _host_inputs(seg_masks, cate_scores, feature_map, x, targets, cate_labels):
    seg = np.asarray(seg_masks, np.float32).reshape(N, -1)
    gfull = (np.asarray(feature_map, np.float32) + 10.0)
    xf = np.asarray(x, np.float32)
    tf = np.asarray(targets, np.float32)
    labels = np.asarray(cate_labels)
    a = np.arange(N)
    maskT = ((a[None, :] < a[:, None]) &
             (labels[:, None] == labels[None, :])).astype(np.float32)
    scol = np.asarray(cate_scores, np.float32).reshape(N, 1)
    sh_p1 = np.eye(H, H, -1, dtype=np.float32)
    sh_m1 = np.eye(H, H, 1, dtype=np.float32)
    ident = np.eye(H, dtype=np.float32)
    ones_row = np.ones((1, H), np.float32)

    in_maps = []
    for c in range(NCORES):
        g = np.zeros((3, H, WT), np.float32)
        xp = np.zeros((H, WT), np.float32)
        tp = np.zeros((H, WT), np.float32)
        for i in range(NI):
            inst = c * NI + i
            g[:, :, _strip(i)] = gfull[inst]
            xp[:, _strip(i)] = xf[inst, 0]
            tp[:, _strip(i)] = tf[inst, 0]
        segT = np.empty((KC, N + 1), ml_dtypes.bfloat16)
        segT[:, :N] = seg[:, c * KC:(c + 1) * KC].T.astype(ml_dtypes.bfloat16)
        segT[:, N] = 1
        in_maps.append({
            "g": g, "xp": xp, "tp": tp, "segT": segT, "maskT": maskT,
            "scores_col": scol, "sh_p1": sh_p1, "sh_m1": sh_m1,
            "ident": ident, "ones_row": ones_row,
        })
    return in_maps


def kernel(seg_masks, cate_scores, feature_map, x, targets, cate_labels,
           _trace=False, _trace_kwargs=None):
    in_maps = _host_inputs(seg_masks, cate_scores, feature_map, x, targets,
                           cate_labels)
    nc = _get_nc()
    res = run_bass_kernel_spmd(nc, in_maps, core_ids=list(range(NCORES)),
                               trace=_trace, **(_trace_kwargs or {}))
    outs = res.results
    masks = np.empty((NCORES * NI, 1, H, W), np.float32)
    valid = np.empty(NCORES * NI, np.float32)
    for c in range(NCORES):
        mc = outs[c]["masks"].reshape(H, NI, W).transpose(1, 0, 2)
        masks[c * NI:(c + 1) * NI, 0] = mc
        valid[c * NI:(c + 1) * NI] = outs[c]["valid"].reshape(NI)
    scores = outs[0]["scores"].reshape(N).astype(np.float32)
    if _trace:
        kernel._last_results = res
    return scores, masks, valid
